# revision 17
# baseline (speedup 1.0000x reference)
"""AL2Loss2d Trainium2 kernel (fp8 DoubleRow edition).

Reference computation:
  inputs [8, 64, 512, 512] f32, targets [8, 512, 512] int64 (values 0..18)
  - per-class sums of the 64-dim pixel features (segment_sum over 2M pixels)
  - per-class counts
  - centers = sums / max(counts, 1); pairwise cosine similarity of the 19
    centers; CosineEmbeddingLoss-style reduction to a scalar.

Strategy: data-parallel over batch, one batch element per NeuronCore.
The rel-err budget (2e-2) is large, so the host ships features as
fp8_e4m3 (measured end-to-end rel err 5.7e-3), quartering HBM traffic
vs f32.

Per-core layout: pixels are packed [128 partitions, 1024 pairs, 2, 65]
fp8; the 65th column holds a per-class code mu[t] (19 distinct,
exactly-representable values), which doubles as the count feature:
accumulator column 64 = mu_k * count_k. Device pipeline per tile:
  - DMA tile (HBM streams ~430 B/ns when not backpressured)
  - DVE builds an 18-class one-hot [128, T, 2, 18] fp8 by byte-comparing
    the code column against the shipped code table (is_equal); class 18
    is recovered on the host from an always-ones 20th stationary column
    whose PSUM row accumulates the unconditional totals (DVE at 1 B/cyc
    is the pipeline's long pole, so shedding 1/19 of its work matters;
    fp16-out 2x variants lose more on the PE side: the scattered
    stationary bytes make dual-fp8 Ldweights 2x slower)
  - TensorE: one DoubleRow fp8 matmul per pixel-pair (256 px / instr,
    0.5 cycles/row) accumulating psum[20, 65]; k-tile step 32 B keeps
    dual-fp8 Ldweights legal (s3_lw_dual_fp8_restrictions: 16B-aligned)
The tiny 19x19 cosine loss runs on host on the 8 gathered partials.
"""

import sys

import ml_dtypes
import numpy as np

if "/opt/trn_rl_repo" not in sys.path:
    sys.path.insert(0, "/opt/trn_rl_repo")

from concourse import bacc, bass, mybir, tile  # noqa: E402
from concourse.bass_utils import run_bass_kernel_spmd  # noqa: E402

K = 19
KOH = 18  # classes built as one-hot; class 18 = totals - rest (host)
CH = 64
CW = CH + 1  # 64 channel sums | mu-scaled count column
KROWS = 20  # PSUM rows: 18 one-hot classes, row 18 unused, row 19 totals
NCORES = 8
NPART = 128
EPS = 1e-8
NPAIR = 1024  # 2048 px per partition = 1024 DoubleRow pairs
NOHBUF = 6  # one-hot ring buffers
PADJ = 1  # pad pair: keeps the HBM partition stride off large pow2 multiples

FP8 = ml_dtypes.float8_e4m3
# 19 distinct per-class codes, all exactly representable in e4m3 so the
# count column mu_k * count_k divides back exactly.
MU = np.array(
    [1, 2, 3, 4, 5, 6, 7, 8, 9, 10, 11, 12, 13, 14, 15, 16, 18, 20, 22],
    dtype=np.float32,
)
MU_FP8 = MU.astype(FP8)
assert np.all(MU_FP8.astype(np.float32) == MU)
MU_BYTES = MU_FP8.view(np.uint8)
assert len(set(MU_BYTES.tolist())) == K


def pair_segments(npair: int, g: int):
    """Fine ramp-up -> main tiles of g pairs -> tapered tail.

    Small leading tiles start the DVE/PE pipeline as soon as the first
    bytes land and keep DVE fed while the DMA queue is still ramping
    (coarse leading tiles starve DVE for several us); small trailing
    tiles shrink the compute left after the last DMA byte.
    """
    ramp = [8, 8, 16, 32, 64]
    tail = [32, 16, 8, 4, 4]
    if npair <= sum(ramp) + sum(tail):
        segs = []
        j = 0
        while j < npair:
            t = min(g, npair - j)
            segs.append((j, t))
            j += t
        return segs
    segs = []
    j = 0
    for t in ramp:
        segs.append((j, t))
        j += t
    while npair - j > sum(tail):
        t = min(g, npair - j - sum(tail))
        segs.append((j, t))
        j += t
    for t in tail:
        segs.append((j, t))
        j += t
    assert sum(s[1] for s in segs) == npair, segs
    return segs


def build(npair: int, g: int) -> bass.Bass:
    """Per-core Bass program (pixels = 128 * npair * 2)."""
    segs = pair_segments(npair, g)
    nc = bacc.Bacc(target_bir_lowering=False, trn_type="TRN2")
    x_ext = nc.declare_dram_parameter(
        "x", [NPART, npair + PADJ, 2, CW], mybir.dt.float8e4, isOutput=False
    )
    mu_ext = nc.declare_dram_parameter(
        "mu", [NPART, 32], mybir.dt.int8, isOutput=False
    )
    out_ext = nc.declare_dram_parameter(
        "out", [KROWS, CW], mybir.dt.float32, isOutput=True
    )

    with tile.TileContext(nc) as tc:
        with (
            tc.tile_pool(name="const", bufs=1) as cpool,
            tc.tile_pool(name="xin", bufs=6) as xpool,
            tc.tile_pool(name="oh", bufs=1) as ohpool,
            tc.tile_pool(name="acc", bufs=1, space=bass.MemorySpace.PSUM) as psumpool,
            tc.tile_pool(name="outp", bufs=1) as opool,
        ):
            # per-class code table, one byte-row per partition (tiny DMA on
            # the Act queue so it never waits behind an x tile)
            mu_sb = cpool.tile([NPART, 32], mybir.dt.int8)
            nc.scalar.dma_start(mu_sb[:], mu_ext[:])

            # Persistent one-hot buffers, rotated manually so the pre-set
            # constant columns survive: col 18 = 0 (dead PSUM row), col 19
            # = 1 (totals row). The per-tile is_equal only writes cols
            # 0..17; WAR deps across rotations are tracked per tensor.
            ohbufs = []
            for i in range(NOHBUF):
                ohb = ohpool.tile(
                    [NPART, g, 2, 32], mybir.dt.float8e4, name=f"ohb{i}"
                )
                # on GpSimd: DVE memsets here would inflate the DVE
                # semaphore threshold every consumer waits on, serializing
                # the first ~13us of the pipeline behind them
                nc.gpsimd.memset(ohb[:, :, :, KOH : KROWS - 1], 0.0)
                nc.gpsimd.memset(ohb[:, :, :, KROWS - 1 : KROWS], 1.0)
                ohbufs.append(ohb)

            acc = psumpool.tile([KROWS, CW], mybir.dt.float32)
            nmm = npair
            mm = 0
            for ti, (j0, gg) in enumerate(segs):
                xt = xpool.tile([NPART, g, 2, CW], mybir.dt.float8e4, tag="xt")
                # ramp tiles go on the Act queue so their ~650ns/DMA
                # HWDGE serialization doesn't delay the first big tiles;
                # all main tiles share one queue so they complete in
                # consumption order (two interleaved queues split HBM
                # bandwidth and starve the in-order DVE consumer)
                deng = nc.scalar if gg < g // 2 and j0 < npair // 2 else nc.sync
                deng.dma_start(xt[:, :gg], x_ext[:, j0 : j0 + gg])
                # one-hot by byte equality of the fp8 class codes. Class
                # pitch is padded to 32 B because the dual-fp8 Ldweights
                # (DoubleRow) requires the k-tile step to be 16B-aligned.
                oh = ohbufs[ti % NOHBUF]
                tcol = (
                    xt[:, :gg, :, CH]
                    .bitcast(mybir.dt.int8)
                    .unsqueeze(3)
                    .broadcast_to([NPART, gg, 2, KOH])
                )
                iob = (
                    mu_sb[:, :KOH]
                    .unsqueeze(1)
                    .unsqueeze(1)
                    .broadcast_to([NPART, gg, 2, KOH])
                )
                nc.vector.tensor_tensor(
                    out=oh[:, :gg, :, :KOH],
                    in0=tcol,
                    in1=iob,
                    op=mybir.AluOpType.is_equal,
                )
                for j in range(gg):
                    nc.tensor.matmul(
                        acc[:],
                        oh[:, j, :, :KROWS],
                        xt[:, j],
                        start=(mm == 0),
                        stop=(mm == nmm - 1),
                        perf_mode=mybir.MatmulPerfMode.DoubleRow,
                    )
                    mm += 1
            out_sb = opool.tile([KROWS, CW], mybir.dt.float32)
            nc.vector.tensor_copy(out_sb[:], acc[:])
            nc.sync.dma_start(out_ext[:], out_sb[:])
    nc.compile()
    return nc


def prep_shard(xq_b: np.ndarray, t_b: np.ndarray, npair: int):
    """xq_b [64, H, W] fp8, t_b [H, W] int -> device arrays."""
    npix = t_b.size
    xr = xq_b.reshape(CH, NPART, npix // NPART).transpose(1, 2, 0)
    xdev = np.zeros((NPART, npair + PADJ, 2, CW), dtype=FP8)
    xdev[:, :npair, :, :CH] = xr.reshape(NPART, npair, 2, CH)
    tcode = MU_FP8[t_b.reshape(NPART, npix // NPART)]
    xdev[:, :npair, :, CH] = tcode.reshape(NPART, npair, 2)
    mudev = np.zeros((NPART, 32), dtype=np.int8)
    mudev[:, :K] = MU_BYTES.view(np.int8)
    return xdev, mudev


_NC_CACHE: dict = {}
TRACE = False  # set True (e.g. from test.py) to profile; result lands here
LAST_RESULT = None
G = 128  # pairs per tile


def _get_nc(npair: int) -> bass.Bass:
    key = (npair, G)
    if key not in _NC_CACHE:
        _NC_CACHE[key] = build(npair, G)
    return _NC_CACHE[key]


def finish(partials: np.ndarray) -> np.float32:
    """partials [ncores, KROWS, CW] -> scalar loss (host, mirrors reference)."""
    total = partials.sum(axis=0, dtype=np.float64)
    sums = total[:KOH, :CH]
    cnt_col = total[:KOH, CH]
    # row 19 holds unconditional totals; class 18 = totals - sum(rest)
    sums_last = total[KROWS - 1, :CH] - sums.sum(axis=0)
    cnt_last = total[KROWS - 1, CH] - cnt_col.sum()
    sums = np.concatenate([sums, sums_last[None, :]], axis=0)
    counts = np.concatenate([cnt_col, [cnt_last]]) / MU.astype(np.float64)
    centers = sums / np.maximum(counts, 1.0)[:, None]
    norms = np.maximum(np.sqrt((centers * centers).sum(axis=1)), EPS)
    cn = centers / norms[:, None]
    S = cn @ cn.T
    eye = np.eye(K, dtype=bool)
    per_pair = np.where(eye, 1.0 - S, np.maximum(S, 0.0))
    return np.float32(per_pair.sum() / (K * K * K))


def kernel(inputs: np.ndarray, targets: np.ndarray) -> np.ndarray:
    B, C, H, W = inputs.shape
    assert (B, C) == (NCORES, CH)
    npair = H * W // NPART // 2
    nc = _get_nc(npair)

    xq = np.asarray(inputs).astype(FP8)
    tgt = np.asarray(targets)
    in_maps = []
    for i in range(NCORES):
        xdev, mudev = prep_shard(xq[i], tgt[i], npair)
        in_maps.append({"x": xdev, "mu": mudev})

    res = run_bass_kernel_spmd(
        nc, in_maps, core_ids=list(range(NCORES)), trace=TRACE
    )
    global LAST_RESULT
    LAST_RESULT = res
    partials = np.stack([r["out"] for r in res.results])
    return np.asarray(finish(partials))


# revision 18
# speedup vs baseline: 1.0687x; 1.0687x over previous
"""AL2Loss2d Trainium2 kernel (fp8 DoubleRow edition).

Reference computation:
  inputs [8, 64, 512, 512] f32, targets [8, 512, 512] int64 (values 0..18)
  - per-class sums of the 64-dim pixel features (segment_sum over 2M pixels)
  - per-class counts
  - centers = sums / max(counts, 1); pairwise cosine similarity of the 19
    centers; CosineEmbeddingLoss-style reduction to a scalar.

Strategy: data-parallel over batch, one batch element per NeuronCore.
The rel-err budget (2e-2) is large, so the host ships features as
fp8_e4m3 (measured end-to-end rel err 5.7e-3), quartering HBM traffic
vs f32.

Per-core layout: pixels are packed [128 partitions, 1024 pairs, 2, 65]
fp8; the 65th column holds a per-class code mu[t] (19 distinct,
exactly-representable values), which doubles as the count feature:
accumulator column 64 = mu_k * count_k. Device pipeline per tile:
  - DMA tile (HBM streams ~430 B/ns when not backpressured)
  - DVE builds an 18-class one-hot [128, T, 2, 18] fp8 by byte-comparing
    the code column against the shipped code table (is_equal); class 18
    is recovered on the host from an always-ones 20th stationary column
    whose PSUM row accumulates the unconditional totals (DVE at 1 B/cyc
    is the pipeline's long pole, so shedding 1/19 of its work matters;
    fp16-out 2x variants lose more on the PE side: the scattered
    stationary bytes make dual-fp8 Ldweights 2x slower)
  - TensorE: one DoubleRow fp8 matmul per pixel-pair (256 px / instr,
    0.5 cycles/row) accumulating psum[20, 65]; k-tile step 32 B keeps
    dual-fp8 Ldweights legal (s3_lw_dual_fp8_restrictions: 16B-aligned)
The tiny 19x19 cosine loss runs on host on the 8 gathered partials.
"""

import sys

import ml_dtypes
import numpy as np

if "/opt/trn_rl_repo" not in sys.path:
    sys.path.insert(0, "/opt/trn_rl_repo")

from concourse import bacc, bass, mybir, tile  # noqa: E402
from concourse.bass_utils import run_bass_kernel_spmd  # noqa: E402

K = 19
KOH = 19  # all classes as one-hot
CH = 64
CW = CH + 1  # 64 channel sums | mu-scaled count column
KROWS = 19  # PSUM rows
NCORES = 8
NPART = 128
EPS = 1e-8
NPAIR = 1024  # 2048 px per partition = 1024 DoubleRow pairs
PADJ = 1  # pad pair: keeps the HBM partition stride off large pow2 multiples

FP8 = ml_dtypes.float8_e4m3
# 19 distinct per-class codes, all exactly representable in e4m3 so the
# count column mu_k * count_k divides back exactly.
MU = np.array(
    [1, 2, 3, 4, 5, 6, 7, 8, 9, 10, 11, 12, 13, 14, 15, 16, 18, 20, 22],
    dtype=np.float32,
)
MU_FP8 = MU.astype(FP8)
assert np.all(MU_FP8.astype(np.float32) == MU)
MU_BYTES = MU_FP8.view(np.uint8)
assert len(set(MU_BYTES.tolist())) == K


def pair_segments(npair: int, g: int):
    """Fine ramp-up -> main tiles of g pairs -> tapered tail.

    Small leading tiles start the DVE/PE pipeline as soon as the first
    bytes land and keep DVE fed while the DMA queue is still ramping
    (coarse leading tiles starve DVE for several us); small trailing
    tiles shrink the compute left after the last DMA byte.
    """
    ramp = [8, 8, 16, 32, 64]
    tail = [32, 16, 8, 4, 4]
    if npair <= sum(ramp) + sum(tail):
        segs = []
        j = 0
        while j < npair:
            t = min(g, npair - j)
            segs.append((j, t))
            j += t
        return segs
    segs = []
    j = 0
    for t in ramp:
        segs.append((j, t))
        j += t
    while npair - j > sum(tail):
        t = min(g, npair - j - sum(tail))
        segs.append((j, t))
        j += t
    for t in tail:
        segs.append((j, t))
        j += t
    assert sum(s[1] for s in segs) == npair, segs
    return segs


def build(npair: int, g: int) -> bass.Bass:
    """Per-core Bass program (pixels = 128 * npair * 2)."""
    segs = pair_segments(npair, g)
    nc = bacc.Bacc(target_bir_lowering=False, trn_type="TRN2")
    x_ext = nc.declare_dram_parameter(
        "x", [NPART, npair + PADJ, 2, CW], mybir.dt.float8e4, isOutput=False
    )
    mu_ext = nc.declare_dram_parameter(
        "mu", [NPART, 32], mybir.dt.int8, isOutput=False
    )
    out_ext = nc.declare_dram_parameter(
        "out", [KROWS, CW], mybir.dt.float32, isOutput=True
    )

    with tile.TileContext(nc) as tc:
        with (
            tc.tile_pool(name="const", bufs=1) as cpool,
            tc.tile_pool(name="xin", bufs=6) as xpool,
            tc.tile_pool(name="oh", bufs=6) as ohpool,
            tc.tile_pool(name="acc", bufs=1, space=bass.MemorySpace.PSUM) as psumpool,
            tc.tile_pool(name="outp", bufs=1) as opool,
        ):
            # per-class code table, one byte-row per partition (tiny DMA on
            # the Act queue so it never waits behind an x tile)
            mu_sb = cpool.tile([NPART, 32], mybir.dt.int8)
            nc.scalar.dma_start(mu_sb[:], mu_ext[:])

            acc = psumpool.tile([KROWS, CW], mybir.dt.float32)
            nmm = npair
            mm = 0
            for ti, (j0, gg) in enumerate(segs):
                xt = xpool.tile([NPART, g, 2, CW], mybir.dt.float8e4, tag="xt")
                # ramp tiles go on the Act queue so their ~650ns/DMA
                # HWDGE serialization doesn't delay the first big tiles;
                # all main tiles share one queue so they complete in
                # consumption order (two interleaved queues split HBM
                # bandwidth and starve the in-order DVE consumer)
                deng = nc.scalar if gg < g // 2 and j0 < npair // 2 else nc.sync
                deng.dma_start(xt[:, :gg], x_ext[:, j0 : j0 + gg])
                # one-hot by byte equality of the fp8 class codes. Class
                # pitch is padded to 32 B because the dual-fp8 Ldweights
                # (DoubleRow) requires the k-tile step to be 16B-aligned.
                oh = ohpool.tile([NPART, g, 2, 32], mybir.dt.float8e4, tag="oh")
                tcol = (
                    xt[:, :gg, :, CH]
                    .bitcast(mybir.dt.int8)
                    .unsqueeze(3)
                    .broadcast_to([NPART, gg, 2, KOH])
                )
                iob = (
                    mu_sb[:, :KOH]
                    .unsqueeze(1)
                    .unsqueeze(1)
                    .broadcast_to([NPART, gg, 2, KOH])
                )
                nc.vector.tensor_tensor(
                    out=oh[:, :gg, :, :KOH],
                    in0=tcol,
                    in1=iob,
                    op=mybir.AluOpType.is_equal,
                )
                for j in range(gg):
                    nc.tensor.matmul(
                        acc[:],
                        oh[:, j, :, :KROWS],
                        xt[:, j],
                        start=(mm == 0),
                        stop=(mm == nmm - 1),
                        perf_mode=mybir.MatmulPerfMode.DoubleRow,
                    )
                    mm += 1
            out_sb = opool.tile([KROWS, CW], mybir.dt.float32)
            nc.vector.tensor_copy(out_sb[:], acc[:])
            nc.sync.dma_start(out_ext[:], out_sb[:])
    nc.compile()
    return nc


def prep_shard(xq_b: np.ndarray, t_b: np.ndarray, npair: int):
    """xq_b [64, H, W] fp8, t_b [H, W] int -> device arrays."""
    npix = t_b.size
    xr = xq_b.reshape(CH, NPART, npix // NPART).transpose(1, 2, 0)
    xdev = np.zeros((NPART, npair + PADJ, 2, CW), dtype=FP8)
    xdev[:, :npair, :, :CH] = xr.reshape(NPART, npair, 2, CH)
    tcode = MU_FP8[t_b.reshape(NPART, npix // NPART)]
    xdev[:, :npair, :, CH] = tcode.reshape(NPART, npair, 2)
    mudev = np.zeros((NPART, 32), dtype=np.int8)
    mudev[:, :K] = MU_BYTES.view(np.int8)
    return xdev, mudev


_NC_CACHE: dict = {}
TRACE = False  # set True (e.g. from test.py) to profile; result lands here
LAST_RESULT = None
G = 128  # pairs per tile


def _get_nc(npair: int) -> bass.Bass:
    key = (npair, G)
    if key not in _NC_CACHE:
        _NC_CACHE[key] = build(npair, G)
    return _NC_CACHE[key]


def finish(partials: np.ndarray) -> np.float32:
    """partials [ncores, KROWS, CW] -> scalar loss (host, mirrors reference)."""
    total = partials.sum(axis=0, dtype=np.float64)
    sums = total[:, :CH]
    counts = total[:, CH] / MU.astype(np.float64)
    centers = sums / np.maximum(counts, 1.0)[:, None]
    norms = np.maximum(np.sqrt((centers * centers).sum(axis=1)), EPS)
    cn = centers / norms[:, None]
    S = cn @ cn.T
    eye = np.eye(K, dtype=bool)
    per_pair = np.where(eye, 1.0 - S, np.maximum(S, 0.0))
    return np.float32(per_pair.sum() / (K * K * K))


def kernel(inputs: np.ndarray, targets: np.ndarray) -> np.ndarray:
    B, C, H, W = inputs.shape
    assert (B, C) == (NCORES, CH)
    npair = H * W // NPART // 2
    nc = _get_nc(npair)

    xq = np.asarray(inputs).astype(FP8)
    tgt = np.asarray(targets)
    in_maps = []
    for i in range(NCORES):
        xdev, mudev = prep_shard(xq[i], tgt[i], npair)
        in_maps.append({"x": xdev, "mu": mudev})

    res = run_bass_kernel_spmd(
        nc, in_maps, core_ids=list(range(NCORES)), trace=TRACE
    )
    global LAST_RESULT
    LAST_RESULT = res
    partials = np.stack([r["out"] for r in res.results])
    return np.asarray(finish(partials))


# revision 19
# speedup vs baseline: 1.1257x; 1.0533x over previous
"""AL2Loss2d Trainium2 kernel (fp8 DoubleRow edition).

Reference computation:
  inputs [8, 64, 512, 512] f32, targets [8, 512, 512] int64 (values 0..18)
  - per-class sums of the 64-dim pixel features (segment_sum over 2M pixels)
  - per-class counts
  - centers = sums / max(counts, 1); pairwise cosine similarity of the 19
    centers; CosineEmbeddingLoss-style reduction to a scalar.

Strategy: data-parallel over batch, one batch element per NeuronCore.
The rel-err budget (2e-2) is large, so the host ships features as
fp8_e4m3 (measured end-to-end rel err 5.7e-3), quartering HBM traffic
vs f32.

Per-core layout: pixels are packed [128 partitions, 1024 pairs, 2, 65]
fp8; the 65th column holds a per-class code mu[t] (19 distinct,
exactly-representable values), which doubles as the count feature:
accumulator column 64 = mu_k * count_k. Device pipeline per tile:
  - DMA tile (HBM streams ~430 B/ns when not backpressured)
  - DVE builds an 18-class one-hot [128, T, 2, 18] fp8 by byte-comparing
    the code column against the shipped code table (is_equal); class 18
    is recovered on the host from an always-ones 20th stationary column
    whose PSUM row accumulates the unconditional totals (DVE at 1 B/cyc
    is the pipeline's long pole, so shedding 1/19 of its work matters;
    fp16-out 2x variants lose more on the PE side: the scattered
    stationary bytes make dual-fp8 Ldweights 2x slower)
  - TensorE: one DoubleRow fp8 matmul per pixel-pair (256 px / instr,
    0.5 cycles/row) accumulating psum[20, 65]; k-tile step 32 B keeps
    dual-fp8 Ldweights legal (s3_lw_dual_fp8_restrictions: 16B-aligned)
The tiny 19x19 cosine loss runs on host on the 8 gathered partials.
"""

import sys

import ml_dtypes
import numpy as np

if "/opt/trn_rl_repo" not in sys.path:
    sys.path.insert(0, "/opt/trn_rl_repo")

from concourse import bacc, bass, mybir, tile  # noqa: E402
from concourse.bass_utils import run_bass_kernel_spmd  # noqa: E402

K = 19
KOH = 19  # all classes as one-hot
CH = 64
CW = CH + 1  # 64 channel sums | mu-scaled count column
KROWS = 19  # PSUM rows
NCORES = 8
NPART = 128
EPS = 1e-8
NPAIR = 1024  # 2048 px per partition = 1024 DoubleRow pairs
PADJ = 1  # pad pair: keeps the HBM partition stride off large pow2 multiples

FP8 = ml_dtypes.float8_e4m3
# 19 distinct per-class codes, all exactly representable in e4m3 so the
# count column mu_k * count_k divides back exactly.
MU = np.array(
    [1, 2, 3, 4, 5, 6, 7, 8, 9, 10, 11, 12, 13, 14, 15, 16, 18, 20, 22],
    dtype=np.float32,
)
MU_FP8 = MU.astype(FP8)
assert np.all(MU_FP8.astype(np.float32) == MU)
MU_BYTES = MU_FP8.view(np.uint8)
assert len(set(MU_BYTES.tolist())) == K


def pair_segments(npair: int, g: int):
    """Fine ramp-up -> main tiles of g pairs -> tapered tail.

    Small leading tiles start the DVE/PE pipeline as soon as the first
    bytes land and keep DVE fed while the DMA queue is still ramping
    (coarse leading tiles starve DVE for several us); small trailing
    tiles shrink the compute left after the last DMA byte.
    """
    ramp = [8, 8, 16, 32, 64]
    tail = [32, 16, 8, 4, 4]
    if npair <= sum(ramp) + sum(tail):
        segs = []
        j = 0
        while j < npair:
            t = min(g, npair - j)
            segs.append((j, t))
            j += t
        return segs
    segs = []
    j = 0
    for t in ramp:
        segs.append((j, t))
        j += t
    while npair - j > sum(tail):
        t = min(g, npair - j - sum(tail))
        segs.append((j, t))
        j += t
    for t in tail:
        segs.append((j, t))
        j += t
    assert sum(s[1] for s in segs) == npair, segs
    return segs


def build(npair: int, g: int) -> bass.Bass:
    """Per-core Bass program (pixels = 128 * npair * 2)."""
    segs = pair_segments(npair, g)
    nc = bacc.Bacc(target_bir_lowering=False, trn_type="TRN2")
    x_ext = nc.declare_dram_parameter(
        "x", [NPART, npair + PADJ, 2, CW], mybir.dt.float8e4, isOutput=False
    )
    mu_ext = nc.declare_dram_parameter(
        "mu", [NPART, 32], mybir.dt.int8, isOutput=False
    )
    out_ext = nc.declare_dram_parameter(
        "out", [KROWS, CW], mybir.dt.float32, isOutput=True
    )

    with tile.TileContext(nc) as tc:
        with (
            tc.tile_pool(name="const", bufs=1) as cpool,
            tc.tile_pool(name="xin", bufs=6) as xpool,
            tc.tile_pool(name="oh", bufs=6) as ohpool,
            tc.tile_pool(name="acc", bufs=1, space=bass.MemorySpace.PSUM) as psumpool,
            tc.tile_pool(name="outp", bufs=1) as opool,
        ):
            # per-class code table, one byte-row per partition (tiny DMA on
            # the Act queue so it never waits behind an x tile)
            mu_sb = cpool.tile([NPART, 32], mybir.dt.int8)
            nc.scalar.dma_start(mu_sb[:], mu_ext[:])

            acc = psumpool.tile([KROWS, CW], mybir.dt.float32)
            nmm = npair
            mm = 0
            for ti, (j0, gg) in enumerate(segs):
                xt = xpool.tile([NPART, g, 2, CW], mybir.dt.float8e4, tag="xt")
                # single queue: tiles must complete in consumption order.
                # A second queue (tried) splits HBM bandwidth and the DMA
                # engines service it last, starving the in-order consumer.
                nc.sync.dma_start(xt[:, :gg], x_ext[:, j0 : j0 + gg])
                # one-hot by byte equality of the fp8 class codes. Class
                # pitch is padded to 32 B because the dual-fp8 Ldweights
                # (DoubleRow) requires the k-tile step to be 16B-aligned.
                oh = ohpool.tile([NPART, g, 2, 32], mybir.dt.float8e4, tag="oh")
                tcol = (
                    xt[:, :gg, :, CH]
                    .bitcast(mybir.dt.int8)
                    .unsqueeze(3)
                    .broadcast_to([NPART, gg, 2, KOH])
                )
                iob = (
                    mu_sb[:, :KOH]
                    .unsqueeze(1)
                    .unsqueeze(1)
                    .broadcast_to([NPART, gg, 2, KOH])
                )
                nc.vector.tensor_tensor(
                    out=oh[:, :gg, :, :KOH],
                    in0=tcol,
                    in1=iob,
                    op=mybir.AluOpType.is_equal,
                )
                for j in range(gg):
                    nc.tensor.matmul(
                        acc[:],
                        oh[:, j, :, :KROWS],
                        xt[:, j],
                        start=(mm == 0),
                        stop=(mm == nmm - 1),
                        perf_mode=mybir.MatmulPerfMode.DoubleRow,
                    )
                    mm += 1
            out_sb = opool.tile([KROWS, CW], mybir.dt.float32)
            nc.vector.tensor_copy(out_sb[:], acc[:])
            nc.sync.dma_start(out_ext[:], out_sb[:])
    nc.compile()
    return nc


def prep_shard(xq_b: np.ndarray, t_b: np.ndarray, npair: int):
    """xq_b [64, H, W] fp8, t_b [H, W] int -> device arrays."""
    npix = t_b.size
    xr = xq_b.reshape(CH, NPART, npix // NPART).transpose(1, 2, 0)
    xdev = np.zeros((NPART, npair + PADJ, 2, CW), dtype=FP8)
    xdev[:, :npair, :, :CH] = xr.reshape(NPART, npair, 2, CH)
    tcode = MU_FP8[t_b.reshape(NPART, npix // NPART)]
    xdev[:, :npair, :, CH] = tcode.reshape(NPART, npair, 2)
    mudev = np.zeros((NPART, 32), dtype=np.int8)
    mudev[:, :K] = MU_BYTES.view(np.int8)
    return xdev, mudev


_NC_CACHE: dict = {}
TRACE = False  # set True (e.g. from test.py) to profile; result lands here
LAST_RESULT = None
G = 128  # pairs per tile


def _get_nc(npair: int) -> bass.Bass:
    key = (npair, G)
    if key not in _NC_CACHE:
        _NC_CACHE[key] = build(npair, G)
    return _NC_CACHE[key]


def finish(partials: np.ndarray) -> np.float32:
    """partials [ncores, KROWS, CW] -> scalar loss (host, mirrors reference)."""
    total = partials.sum(axis=0, dtype=np.float64)
    sums = total[:, :CH]
    counts = total[:, CH] / MU.astype(np.float64)
    centers = sums / np.maximum(counts, 1.0)[:, None]
    norms = np.maximum(np.sqrt((centers * centers).sum(axis=1)), EPS)
    cn = centers / norms[:, None]
    S = cn @ cn.T
    eye = np.eye(K, dtype=bool)
    per_pair = np.where(eye, 1.0 - S, np.maximum(S, 0.0))
    return np.float32(per_pair.sum() / (K * K * K))


def kernel(inputs: np.ndarray, targets: np.ndarray) -> np.ndarray:
    B, C, H, W = inputs.shape
    assert (B, C) == (NCORES, CH)
    npair = H * W // NPART // 2
    nc = _get_nc(npair)

    xq = np.asarray(inputs).astype(FP8)
    tgt = np.asarray(targets)
    in_maps = []
    for i in range(NCORES):
        xdev, mudev = prep_shard(xq[i], tgt[i], npair)
        in_maps.append({"x": xdev, "mu": mudev})

    res = run_bass_kernel_spmd(
        nc, in_maps, core_ids=list(range(NCORES)), trace=TRACE
    )
    global LAST_RESULT
    LAST_RESULT = res
    partials = np.stack([r["out"] for r in res.results])
    return np.asarray(finish(partials))
